# revision 24
# baseline (speedup 1.0000x reference)
"""Graphormer attention (N=2048, D=512, H=8 heads of 64) on 8 NeuronCores.

Strategy (tensor-parallel over heads, one head per core):
  - Host packs x^T + all per-head weights into ONE contiguous DRAM image so
    startup needs 5 big DMA triggers instead of 26 small ones (each trigger
    costs ~600ns of serial sync-queue time).
  - The z-bin bias is folded in multiplicatively: host precomputes
    W = exp(z_table[bin(z)]) in the kernel's [key, query] layout, pre-packed
    per query-chunk so each DMA is contiguous (4KB/partition lines).
  - The K-projection bias bk only adds a per-query constant to scores
    (q . bk), which softmax cancels exactly -> dropped. bq is kept (its
    bq . k_m term varies across keys). SCALE is folded into Wq/bq on host.
  - On device (per core): fused Q^T/K^T projection (one [128,128] weight
    block -> Q rows 0:64, K rows 64:128), evacuated per 512-col chunk with
    the two evacs split across ScalarE/VectorE so S can start early.
  - Main loop per k-tile (baseline's proven in-order pattern -- S(t+2)
    emitted before exp/mult/PV(t) so a stalled PV never starves the next
    exp): S^T = K^T x Q^T (fp32, PSUM) -> exp on ScalarE (the bottleneck:
    32 x [128,1024] @ ~1.0us) -> P = exp(S)*W on VectorE -> O'^T +=
    V'[128,65] x P (65th V column = ones => row 64 of O' = softmax denom Z).
  - V projection runs through the ps_o PSUM bank before the k-tile loop;
    QK half-1 is interleaved mid-loop (DVE evacs; ACT stays exp-saturated);
    Y^T(qc=0) projection blocks hide inside qc=1's loop shadow.
  - Host divides each head's partial Y by its Z, sums heads, adds biases.
"""

import numpy as np
import ml_dtypes
from contextlib import ExitStack

import concourse.bass as bass
import concourse.tile as tile
from concourse import bacc, mybir
from concourse import bass_utils

N = 2048
D = 512
H = 8
HD = 64
NUM_Z_BINS = 16
MAX_Z = 5.0
SCALE = HD ** -0.5
NCORES = 8
QL = 1024          # query-chunk length (PSUM budget)
QC = N // QL       # 2 query chunks
KT = N // 128      # 16 key tiles
CH = D // 128      # 4 contraction chunks of the model dim

# packed image column offsets (bf16, [128, IMG_COLS])
XO = 0             # x^T, col-chunked: [j, c, u] -> j*2048 + c*512 + u
WQKO = 8192        # [Wq*SCALE | Wk] chunks: c*128 + v
BQO = 8704         # bq*SCALE, rows 0:64 (avoids a tiny separate DMA whose
                   # 4-byte descriptors complete ~10us late behind big ones)
WVO = 8708         # Wv chunks: c*64 + v
WOO = 8964         # Wo rows 0:64 (rows 64:128 unused)
IMG_COLS = 9476

FP32 = mybir.dt.float32
FP16 = mybir.dt.float16
BF16 = mybir.dt.bfloat16
BF16_NP = ml_dtypes.bfloat16
FP16_NP = np.float16

AF = mybir.ActivationFunctionType
OP = mybir.AluOpType

_PROGRAM_CACHE = {}


def _build_program():
    if "nc" in _PROGRAM_CACHE:
        return _PROGRAM_CACHE["nc"]

    nc = bacc.Bacc(
        "TRN2",
        target_bir_lowering=False,
        debug=False,
        enable_asserts=False,
        num_devices=NCORES,
    )

    img = nc.dram_tensor("img", [128, IMG_COLS], BF16, kind="ExternalInput").ap()
    wt = nc.dram_tensor("wt", [QC * 128, KT * 1024], FP16, kind="ExternalInput").ap()

    ypT = nc.dram_tensor("ypT", [D, N], FP16, kind="ExternalOutput").ap()
    zrow = nc.dram_tensor("zrow", [N], FP16, kind="ExternalOutput").ap()

    with tile.TileContext(nc) as tc:
        with ExitStack() as ctx:
            _emit(ctx, tc, img, wt, ypT, zrow)
    nc.compile()
    _PROGRAM_CACHE["nc"] = nc
    return nc


def _emit(ctx, tc, img, wt, ypT, zrow):
    nc = tc.nc

    singles = ctx.enter_context(tc.tile_pool(name="singles", bufs=1))
    # PSUM: ps_a 3 x [128,1024]f32 (2 banks each) shared by warmup/qk/s/y;
    # ps_o 1 x [128,1024]f32 (2 banks) holds V' then the O' accumulator.
    ps_a = ctx.enter_context(tc.tile_pool(name="ps_a", bufs=3, space="PSUM"))
    ps_o = ctx.enter_context(tc.tile_pool(name="ps_o", bufs=1, space="PSUM"))
    wpool = ctx.enter_context(tc.tile_pool(name="wpool", bufs=4))
    epool = ctx.enter_context(tc.tile_pool(name="epool", bufs=4))
    ppool = ctx.enter_context(tc.tile_pool(name="ppool", bufs=3))
    ypool = ctx.enter_context(tc.tile_pool(name="ypool", bufs=4))

    w_tiles = {}

    def emit_w(qc, i):
        w = wpool.tile([128, 2048], FP16, tag="w")
        nc.sync.dma_start(
            out=w, in_=wt[qc * 128:(qc + 1) * 128, i * 2048:(i + 1) * 2048])
        w_tiles[(qc, i)] = w

    # ---- input DMAs. The QK weights ride the Scalar queue (also HWDGE)
    # so they stream concurrently with the x^T chunks on the Sync queue;
    # order within each queue = completion priority (exp stream needs
    # wqk+bias, j0, j1 first).
    img_sb = singles.tile([128, IMG_COLS], BF16)
    nc.scalar.dma_start(out=img_sb[:, WQKO:WVO], in_=img[:, WQKO:WVO])
    nc.sync.dma_start(out=img_sb[:, 0:2048], in_=img[:, 0:2048])
    nc.scalar.dma_start(out=img_sb[:, WVO:IMG_COLS], in_=img[:, WVO:IMG_COLS])
    nc.sync.dma_start(out=img_sb[:, 2048:4096], in_=img[:, 2048:4096])
    nc.sync.dma_start(out=img_sb[:, 4096:6144], in_=img[:, 4096:6144])
    nc.sync.dma_start(out=img_sb[:, 6144:8192], in_=img[:, 6144:8192])
    emit_w(0, 0)
    emit_w(0, 1)
    emit_w(0, 2)
    emit_w(0, 3)
    bq_sb = singles.tile([HD, 1], FP32)
    nc.vector.tensor_copy(bq_sb, img_sb[0:64, BQO:BQO + 1])

    # ---- warmup: prime the ACT table load + keep the PE HAM busy -------
    scratch = singles.tile([128, 512], BF16)
    nc.vector.memset(scratch, 0.0)
    v_sb = singles.tile([128, KT * (HD + 1)], FP16)
    nc.vector.memset(v_sb, 1.0)  # col 64 of each V' tile stays 1.0 (Z row)
    warm16 = singles.tile([1, 16], FP16)
    nc.scalar.activation(warm16, scratch[0:1, 0:16], AF.Exp)  # table load now
    wu = ps_a.tile([128, 1024], FP32, tag="big")
    for _ in range(4):
        nc.tensor.matmul(wu[:, 0:512], lhsT=scratch[:, 0:128], rhs=scratch,
                         start=True, stop=True)

    # q2/k2 hold Q^T/K^T duplicated in BOTH partition halves so the K=64
    # S^T matmuls can ROW-TILE: even k-tiles use PE rows 0:63, odd tiles
    # rows 64:127 -- adjacent tiles execute concurrently when the PE is the
    # laggard (i.e. exactly in the clock-throttled regime).
    q2 = singles.tile([128, N], BF16)
    k2 = singles.tile([128, N], BF16)
    oT_sb = singles.tile([HD + 1, N], FP16)  # rows 0:65 = [O'; Z]
    oB_sb = singles.tile([128, N], FP16)     # rows 64:128 = O' copy (Y pairs)

    def emit_qk_half(half, startup):
        pt = ps_a.tile([128, 1024], FP32, tag="big")
        for n_ in range(2):
            j = half * 2 + n_
            for c in range(CH):
                nc.tensor.matmul(
                    pt[:, n_ * 512:(n_ + 1) * 512],
                    lhsT=img_sb[:, WQKO + c * 128:WQKO + (c + 1) * 128],
                    rhs=img_sb[:, XO + j * 2048 + c * 512:XO + j * 2048 + (c + 1) * 512],
                    start=(c == 0),
                    stop=(c == CH - 1),
                )
            if startup:
                # 512-col granularity, split across both engines so the
                # first S matmuls can start as early as possible.
                src = slice(n_ * 512, (n_ + 1) * 512)
                dst = slice(half * 1024 + n_ * 512, half * 1024 + (n_ + 1) * 512)
                nc.vector.tensor_scalar(q2[0:64, dst], pt[0:64, src], bq_sb,
                                        None, OP.add)
                nc.vector.tensor_scalar(q2[64:128, dst], pt[0:64, src], bq_sb,
                                        None, OP.add)
                nc.scalar.copy(k2[64:128, dst], pt[64:128, src])
                nc.scalar.copy(k2[0:64, dst], pt[64:128, src])
        if not startup:
            # mid-loop: ACT is exp-saturated -> evacs on DVE; the partition-
            # half duplicates ride cheap SBUF->SBUF DMAs off the sync queue.
            dst = slice(half * 1024, (half + 1) * 1024)
            nc.vector.tensor_scalar(q2[0:64, dst], pt[0:64, :], bq_sb,
                                    None, OP.add)
            nc.vector.tensor_copy(k2[64:128, dst], pt[64:128, :])
            nc.sync.dma_start(out=q2[64:128, dst], in_=q2[0:64, dst])
            nc.sync.dma_start(out=k2[0:64, dst], in_=k2[64:128, dst])

    emit_qk_half(0, startup=True)

    # ---- main-loop emission helpers ------------------------------------
    s_tiles = {}

    def emit_s_pair(qc, t0):
        # Interleave the two tiles' 512-chunks so their row groups overlap.
        sa = ps_a.tile([128, 1024], FP32, tag="big")
        sb = ps_a.tile([128, 1024], FP32, tag="big")
        for n_ in range(2):
            nc.tensor.matmul(
                sa[:, n_ * 512:(n_ + 1) * 512],
                lhsT=k2[0:64, t0 * 128:(t0 + 1) * 128],
                rhs=q2[0:64, qc * QL + n_ * 512:qc * QL + (n_ + 1) * 512],
                start=True,
                stop=True,
            )
            nc.tensor.matmul(
                sb[:, n_ * 512:(n_ + 1) * 512],
                lhsT=k2[64:128, (t0 + 1) * 128:(t0 + 2) * 128],
                rhs=q2[64:128, qc * QL + n_ * 512:qc * QL + (n_ + 1) * 512],
                start=True,
                stop=True,
            )
        s_tiles[(qc, t0)] = sa
        s_tiles[(qc, t0 + 1)] = sb

    def emit_xpv(qc, t, ot):
        st = s_tiles.pop((qc, t))
        e = epool.tile([128, 1024], FP16, tag="e")
        nc.scalar.activation(e, st, AF.Exp)
        p = ppool.tile([128, 1024], FP16, tag="p")
        w = w_tiles[(qc, t // 2)]
        nc.vector.tensor_mul(p, e, w[:, (t % 2) * 1024:(t % 2 + 1) * 1024])
        for n_ in range(2):
            nc.tensor.matmul(
                ot[0:HD + 1, n_ * 512:(n_ + 1) * 512],
                lhsT=v_sb[:, t * (HD + 1):(t + 1) * (HD + 1)],
                rhs=p[:, n_ * 512:(n_ + 1) * 512],
                start=(t == 0),
                stop=(t == KT - 1),
            )

    def emit_y_pair(n2, m0, evac):
        # Row-tiled pair: block m0 on PE rows 0:63, block m0+1 on 64:127,
        # chunk-interleaved so the two blocks execute concurrently.
        ya = ps_a.tile([128, 1024], FP32, tag="big")
        yb = ps_a.tile([128, 1024], FP32, tag="big")
        for n_ in range(2):
            cols = slice(n2 * QL + n_ * 512, n2 * QL + (n_ + 1) * 512)
            nc.tensor.matmul(
                ya[:, n_ * 512:(n_ + 1) * 512],
                lhsT=img_sb[0:64, WOO + m0 * 128:WOO + (m0 + 1) * 128],
                rhs=oT_sb[0:64, cols],
                start=True,
                stop=True,
            )
            nc.tensor.matmul(
                yb[:, n_ * 512:(n_ + 1) * 512],
                lhsT=img_sb[64:128, WOO + (m0 + 1) * 128:WOO + (m0 + 2) * 128],
                rhs=oB_sb[64:128, cols],
                start=True,
                stop=True,
            )
        for m, yt in ((m0, ya), (m0 + 1, yb)):
            y_sb = ypool.tile([128, 1024], FP16, tag="ysb")
            if evac == "v":
                nc.vector.tensor_copy(y_sb, yt)
            else:
                nc.scalar.copy(y_sb[:, 0:512], yt[:, 0:512])
                nc.vector.tensor_copy(y_sb[:, 512:1024], yt[:, 512:1024])
            nc.sync.dma_start(
                out=ypT[m * 128:(m + 1) * 128, n2 * QL:(n2 + 1) * QL], in_=y_sb)

    # ---- qc = 0 ---------------------------------------------------------
    emit_s_pair(0, 0)
    # V': [k-tile 128, 65] per tile (col 64 = ones). Uses the ps_o bank so
    # the S-slot rotation keeps 3 buffers.
    vp = ps_o.tile([128, 1024], FP32, tag="o")
    for m in range(KT):
        for c in range(CH):
            base = XO + (m // 4) * 2048 + c * 512 + (m % 4) * 128
            nc.tensor.matmul(
                vp[:, m * HD:(m + 1) * HD],
                lhsT=img_sb[:, base:base + 128],
                rhs=img_sb[:, WVO + c * HD:WVO + (c + 1) * HD],
                start=(c == 0),
                stop=(c == CH - 1),
            )
    nc.vector.tensor_copy(
        v_sb.rearrange("p (t c) -> p t c", c=HD + 1)[:, :, 0:HD],
        vp.rearrange("p (t c) -> p t c", c=HD),
    )
    ot = ps_o.tile([128, 1024], FP32, tag="o")
    for t in range(KT):
        if t % 2 == 0:
            if t + 2 < KT:
                emit_s_pair(0, t + 2)
            else:
                emit_s_pair(1, 0)   # hoist qc=1's first S pair so the exp
                                    # stream crosses the qc boundary cleanly
        if t in (4, 6, 8, 10):
            emit_w(0, t // 2 + 2)
        if t in (12, 14):
            emit_w(1, t // 2 - 6)
        emit_xpv(0, t, ot)
        if t == 4:
            emit_qk_half(1, startup=False)
    # O'(qc=0) -> SBUF on DVE (ACT stays exp-saturated); dup for Y pairs.
    nc.vector.tensor_copy(oT_sb[:, 0:QL], ot[0:HD + 1, :])
    nc.vector.tensor_copy(oB_sb[64:128, 0:QL], oT_sb[0:64, 0:QL])
    nc.sync.dma_start(out=zrow.rearrange("(a n) -> a n", a=1)[:, 0:QL],
                      in_=oT_sb[HD:HD + 1, 0:QL])

    # ---- qc = 1 (Y pairs of qc=0 hidden in this loop's shadow) ----------
    ot = ps_o.tile([128, 1024], FP32, tag="o")
    for t in range(KT):
        if t % 2 == 0 and t + 2 < KT:
            emit_s_pair(1, t + 2)
        if t in (2, 4, 6, 8, 10, 12):
            emit_w(1, t // 2 + 1)
        emit_xpv(1, t, ot)
        if t == 4:
            emit_y_pair(0, 0, evac="v")
        if t == 10:
            emit_y_pair(0, 2, evac="v")

    # ---- tail: O'(qc=1) evac split across both engines at 512-col
    # granularity so the first Y matmuls start half a copy earlier --------
    nc.scalar.copy(oT_sb[:, QL:QL + 512], ot[0:HD + 1, 0:512])
    nc.vector.tensor_copy(oT_sb[:, QL + 512:N], ot[0:HD + 1, 512:1024])
    nc.vector.tensor_copy(oB_sb[64:128, QL:N], oT_sb[0:64, QL:N])
    nc.sync.dma_start(out=zrow.rearrange("(a n) -> a n", a=1)[:, QL:N],
                      in_=oT_sb[HD:HD + 1, QL:N])
    emit_y_pair(1, 0, evac="sv")
    emit_y_pair(1, 2, evac="sv")


def _install_ntff_hook():
    """Recreate the missing ``antenv.axon_hooks`` module so that
    run_bass_kernel_spmd(trace=True) can capture NTFF profiles via the
    libaxon_pjrt.so ctypes hook (see trn_agent_boot.trn_boot)."""
    import sys
    import types

    try:
        import antenv.axon_hooks  # noqa: F401
        return
    except ImportError:
        pass
    import antenv
    from trn_agent_boot.trn_boot import _ntff_profile_via_ctypes

    mod = types.ModuleType("antenv.axon_hooks")
    mod._hook = _ntff_profile_via_ctypes("/opt/axon/libaxon_pjrt.so")
    mod.set_axon_ntff_profile_hook = lambda h: setattr(mod, "_hook", h)
    mod.get_axon_ntff_profile_hook = lambda: mod._hook
    sys.modules["antenv.axon_hooks"] = mod
    antenv.axon_hooks = mod
    # keep profile artifacts local; the sandbox has no bucket access
    bass_utils.upload_artifacts = lambda tmpdir: tmpdir


def kernel(x, z_matrix, Wq, bq, Wk, bk, Wv, bv, Wo, bo, z_table, _trace=False):
    if _trace:
        _install_ntff_hook()
    x = np.ascontiguousarray(np.asarray(x, dtype=np.float32))
    z_matrix = np.asarray(z_matrix, dtype=np.float32)
    Wq = np.asarray(Wq, dtype=np.float32)
    Wk = np.asarray(Wk, dtype=np.float32)
    Wv = np.asarray(Wv, dtype=np.float32)
    Wo = np.asarray(Wo, dtype=np.float32)
    bq = np.asarray(bq, dtype=np.float32)
    bk = np.asarray(bk, dtype=np.float32)  # cancels in softmax; unused
    bv = np.asarray(bv, dtype=np.float32)
    bo = np.asarray(bo, dtype=np.float32)
    z_table = np.asarray(z_table, dtype=np.float32)

    nc = _build_program()

    xT_img = (
        x.T.astype(BF16_NP)
        .reshape(CH, 128, 4, 512)
        .transpose(1, 2, 0, 3)
        .reshape(128, 8192)
    )
    binsT = np.clip(
        np.floor(z_matrix.T / MAX_Z * NUM_Z_BINS).astype(np.int32), 0, NUM_Z_BINS - 1
    )
    exp_tab = np.exp(z_table)  # [16, H] fp32

    in_maps = []
    for h in range(NCORES):
        sl = slice(h * HD, (h + 1) * HD)
        img = np.zeros((128, IMG_COLS), dtype=BF16_NP)
        img[:, XO:XO + 8192] = xT_img
        wqk = np.concatenate([Wq[:, sl] * SCALE, Wk[:, sl]], axis=1)
        img[:, WQKO:WQKO + 512] = (
            wqk.astype(BF16_NP).reshape(CH, 128, 128)
            .transpose(1, 0, 2).reshape(128, 512)
        )
        img[:, WVO:WVO + 256] = (
            Wv[:, sl].astype(BF16_NP).reshape(CH, 128, HD)
            .transpose(1, 0, 2).reshape(128, 256)
        )
        img[0:64, WOO:WOO + 512] = Wo[sl].astype(BF16_NP)
        img[64:128, WOO:WOO + 512] = Wo[sl].astype(BF16_NP)
        img[0:64, BQO] = (bq[sl] * SCALE).astype(BF16_NP)
        wt_h = exp_tab[:, h][binsT].astype(FP16_NP)  # [key, query] layout
        wt_in = (
            wt_h.reshape(KT, 128, QC, 1024)
            .transpose(2, 1, 0, 3).reshape(QC * 128, KT * 1024)
        )
        in_maps.append({
            "img": img,
            "wt": np.ascontiguousarray(wt_in),
        })

    res = bass_utils.run_bass_kernel_spmd(
        nc, in_maps, core_ids=list(range(NCORES)), trace=_trace,
    )

    acc = np.zeros((D, N), dtype=np.float64)
    for h in range(NCORES):
        ypT_h = res.results[h]["ypT"].astype(np.float64)
        z_h = res.results[h]["zrow"].astype(np.float64)
        acc += ypT_h / z_h[None, :]
    out = acc.T + (bv @ Wo)[None, :] + bo[None, :]
    out_f32 = out.astype(np.float32)
    if _trace:
        return out_f32, res
    return out_f32


# revision 27
# speedup vs baseline: 1.1073x; 1.1073x over previous
"""Graphormer attention (N=2048, D=512, H=8 heads of 64) on 8 NeuronCores.

Strategy (tensor-parallel over heads, one head per core):
  - Host packs x^T + all per-head weights into ONE contiguous DRAM image so
    startup needs 5 big DMA triggers instead of 26 small ones (each trigger
    costs ~600ns of serial sync-queue time).
  - The z-bin bias is folded in multiplicatively: host precomputes
    W = exp(z_table[bin(z)]) in the kernel's [key, query] layout, pre-packed
    per query-chunk so each DMA is contiguous (4KB/partition lines).
  - The K-projection bias bk only adds a per-query constant to scores
    (q . bk), which softmax cancels exactly -> dropped. bq is kept (its
    bq . k_m term varies across keys). SCALE is folded into Wq/bq on host.
  - On device (per core): fused Q^T/K^T projection (one [128,128] weight
    block -> Q rows 0:64, K rows 64:128), evacuated per 512-col chunk with
    the two evacs split across ScalarE/VectorE so S can start early.
  - Main loop per k-tile (baseline's proven in-order pattern -- S(t+2)
    emitted before exp/mult/PV(t) so a stalled PV never starves the next
    exp): S^T = K^T x Q^T (fp32, PSUM) -> exp on ScalarE (the bottleneck:
    32 x [128,1024] @ ~1.0us) -> P = exp(S)*W on VectorE -> O'^T +=
    V'[128,65] x P (65th V column = ones => row 64 of O' = softmax denom Z).
  - V projection runs through the ps_o PSUM bank before the k-tile loop;
    QK half-1 is interleaved mid-loop (DVE evacs; ACT stays exp-saturated);
    Y^T(qc=0) projection blocks hide inside qc=1's loop shadow.
  - Host divides each head's partial Y by its Z, sums heads, adds biases.
"""

import numpy as np
import ml_dtypes
from contextlib import ExitStack

import concourse.bass as bass
import concourse.tile as tile
from concourse import bacc, mybir
from concourse import bass_utils

N = 2048
D = 512
H = 8
HD = 64
NUM_Z_BINS = 16
MAX_Z = 5.0
SCALE = HD ** -0.5
NCORES = 8
QL = 1024          # query-chunk length (PSUM budget)
QC = N // QL       # 2 query chunks
KT = N // 128      # 16 key tiles
CH = D // 128      # 4 contraction chunks of the model dim

# packed image column offsets (bf16, [128, IMG_COLS])
XO = 0             # x^T, col-chunked: [j, c, u] -> j*2048 + c*512 + u
WQKO = 8192        # [Wq*SCALE | Wk] chunks: c*128 + v
BQO = 8704         # bq*SCALE, rows 0:64 (avoids a tiny separate DMA whose
                   # 4-byte descriptors complete ~10us late behind big ones)
WVO = 8708         # Wv chunks: c*64 + v
WOO = 8964         # Wo rows 0:64 (rows 64:128 unused)
IMG_COLS = 9476

FP32 = mybir.dt.float32
FP16 = mybir.dt.float16
BF16 = mybir.dt.bfloat16
BF16_NP = ml_dtypes.bfloat16
FP16_NP = np.float16

AF = mybir.ActivationFunctionType
OP = mybir.AluOpType

_PROGRAM_CACHE = {}


def _build_program():
    if "nc" in _PROGRAM_CACHE:
        return _PROGRAM_CACHE["nc"]

    nc = bacc.Bacc(
        "TRN2",
        target_bir_lowering=False,
        debug=False,
        enable_asserts=False,
        num_devices=NCORES,
    )

    img = nc.dram_tensor("img", [128, IMG_COLS], BF16, kind="ExternalInput").ap()
    wt = nc.dram_tensor("wt", [QC * 128, KT * 1024], FP16, kind="ExternalInput").ap()

    ypT = nc.dram_tensor("ypT", [D, N], FP16, kind="ExternalOutput").ap()
    zrow = nc.dram_tensor("zrow", [N], FP16, kind="ExternalOutput").ap()

    with tile.TileContext(nc) as tc:
        with ExitStack() as ctx:
            _emit(ctx, tc, img, wt, ypT, zrow)
    nc.compile()
    _PROGRAM_CACHE["nc"] = nc
    return nc


def _emit(ctx, tc, img, wt, ypT, zrow):
    nc = tc.nc

    singles = ctx.enter_context(tc.tile_pool(name="singles", bufs=1))
    # PSUM: ps_a 3 x [128,1024]f32 (2 banks each) shared by warmup/qk/s/y;
    # ps_o 1 x [128,1024]f32 (2 banks) holds V' then the O' accumulator.
    ps_a = ctx.enter_context(tc.tile_pool(name="ps_a", bufs=3, space="PSUM"))
    ps_o = ctx.enter_context(tc.tile_pool(name="ps_o", bufs=1, space="PSUM"))
    wpool = ctx.enter_context(tc.tile_pool(name="wpool", bufs=4))
    epool = ctx.enter_context(tc.tile_pool(name="epool", bufs=4))
    ppool = ctx.enter_context(tc.tile_pool(name="ppool", bufs=3))
    ypool = ctx.enter_context(tc.tile_pool(name="ypool", bufs=4))

    w_tiles = {}

    def emit_w(qc, i):
        w = wpool.tile([128, 2048], FP16, tag="w")
        nc.sync.dma_start(
            out=w, in_=wt[qc * 128:(qc + 1) * 128, i * 2048:(i + 1) * 2048])
        w_tiles[(qc, i)] = w

    # ---- input DMAs. The QK weights ride the Scalar queue (also HWDGE)
    # so they stream concurrently with the x^T chunks on the Sync queue;
    # order within each queue = completion priority (exp stream needs
    # wqk+bias, j0, j1 first).
    img_sb = singles.tile([128, IMG_COLS], BF16)
    nc.scalar.dma_start(out=img_sb[:, WQKO:WVO], in_=img[:, WQKO:WVO])
    nc.sync.dma_start(out=img_sb[:, 0:2048], in_=img[:, 0:2048])
    nc.scalar.dma_start(out=img_sb[:, WVO:IMG_COLS], in_=img[:, WVO:IMG_COLS])
    nc.sync.dma_start(out=img_sb[:, 2048:4096], in_=img[:, 2048:4096])
    nc.sync.dma_start(out=img_sb[:, 4096:6144], in_=img[:, 4096:6144])
    nc.sync.dma_start(out=img_sb[:, 6144:8192], in_=img[:, 6144:8192])
    emit_w(0, 0)
    emit_w(0, 1)
    emit_w(0, 2)
    emit_w(0, 3)
    bq_sb = singles.tile([HD, 1], FP32)
    nc.vector.tensor_copy(bq_sb, img_sb[0:64, BQO:BQO + 1])

    # ---- warmup: prime the ACT table load + keep the PE HAM busy -------
    scratch = singles.tile([128, 512], BF16)
    nc.vector.memset(scratch, 0.0)
    v_sb = singles.tile([128, KT * (HD + 1)], FP16)
    nc.vector.memset(v_sb, 1.0)  # col 64 of each V' tile stays 1.0 (Z row)
    warm16 = singles.tile([1, 16], FP16)
    nc.scalar.activation(warm16, scratch[0:1, 0:16], AF.Exp)  # table load now
    wu = ps_a.tile([128, 1024], FP32, tag="big")
    for _ in range(4):
        nc.tensor.matmul(wu[:, 0:512], lhsT=scratch[:, 0:128], rhs=scratch,
                         start=True, stop=True)

    # q2/k2 hold Q^T/K^T duplicated in BOTH partition halves so the K=64
    # S^T matmuls can ROW-TILE: even k-tiles use PE rows 0:63, odd tiles
    # rows 64:127 -- adjacent tiles execute concurrently when the PE is the
    # laggard (i.e. exactly in the clock-throttled regime).
    q2 = singles.tile([128, N], BF16)
    k2 = singles.tile([128, N], BF16)
    oT_sb = singles.tile([HD + 1, N], FP16)  # rows 0:65 = [O'; Z]
    oB_sb = singles.tile([128, N], FP16)     # rows 64:128 = O' copy (Y pairs)

    def emit_qk_half(half, startup):
        pt = ps_a.tile([128, 1024], FP32, tag="big")
        for n_ in range(2):
            j = half * 2 + n_
            for c in range(CH):
                nc.tensor.matmul(
                    pt[:, n_ * 512:(n_ + 1) * 512],
                    lhsT=img_sb[:, WQKO + c * 128:WQKO + (c + 1) * 128],
                    rhs=img_sb[:, XO + j * 2048 + c * 512:XO + j * 2048 + (c + 1) * 512],
                    start=(c == 0),
                    stop=(c == CH - 1),
                )
            if startup:
                # 512-col granularity, split across both engines so the
                # first S matmuls can start as early as possible.
                src = slice(n_ * 512, (n_ + 1) * 512)
                dst = slice(half * 1024 + n_ * 512, half * 1024 + (n_ + 1) * 512)
                # lo-half copies first on both engines: S(0) runs on the lo
                # half, so q-lo/k-lo gate the first exp.
                nc.vector.tensor_scalar(q2[0:64, dst], pt[0:64, src], bq_sb,
                                        None, OP.add)
                nc.scalar.copy(k2[0:64, dst], pt[64:128, src])
                nc.vector.tensor_scalar(q2[64:128, dst], pt[0:64, src], bq_sb,
                                        None, OP.add)
                nc.scalar.copy(k2[64:128, dst], pt[64:128, src])
        if not startup:
            # mid-loop: ACT is exp-saturated -> evacs on DVE; the partition-
            # half duplicates ride cheap SBUF->SBUF DMAs off the sync queue.
            dst = slice(half * 1024, (half + 1) * 1024)
            nc.vector.tensor_scalar(q2[0:64, dst], pt[0:64, :], bq_sb,
                                    None, OP.add)
            nc.vector.tensor_copy(k2[64:128, dst], pt[64:128, :])
            nc.sync.dma_start(out=q2[64:128, dst], in_=q2[0:64, dst])
            nc.sync.dma_start(out=k2[0:64, dst], in_=k2[64:128, dst])

    emit_qk_half(0, startup=True)

    # ---- main-loop emission helpers ------------------------------------
    s_tiles = {}

    def emit_s_pair(qc, t0):
        # Interleave the two tiles' 512-chunks so their row groups overlap.
        sa = ps_a.tile([128, 1024], FP32, tag="big")
        sb = ps_a.tile([128, 1024], FP32, tag="big")
        for n_ in range(2):
            nc.tensor.matmul(
                sa[:, n_ * 512:(n_ + 1) * 512],
                lhsT=k2[0:64, t0 * 128:(t0 + 1) * 128],
                rhs=q2[0:64, qc * QL + n_ * 512:qc * QL + (n_ + 1) * 512],
                start=True,
                stop=True,
            )
            nc.tensor.matmul(
                sb[:, n_ * 512:(n_ + 1) * 512],
                lhsT=k2[64:128, (t0 + 1) * 128:(t0 + 2) * 128],
                rhs=q2[64:128, qc * QL + n_ * 512:qc * QL + (n_ + 1) * 512],
                start=True,
                stop=True,
            )
        s_tiles[(qc, t0)] = sa
        s_tiles[(qc, t0 + 1)] = sb

    def emit_xpv(qc, t, ot):
        st = s_tiles.pop((qc, t))
        e = epool.tile([128, 1024], FP16, tag="e")
        nc.scalar.activation(e, st, AF.Exp)
        p = ppool.tile([128, 1024], FP16, tag="p")
        w = w_tiles[(qc, t // 2)]
        nc.vector.tensor_mul(p, e, w[:, (t % 2) * 1024:(t % 2 + 1) * 1024])
        for n_ in range(2):
            nc.tensor.matmul(
                ot[0:HD + 1, n_ * 512:(n_ + 1) * 512],
                lhsT=v_sb[:, t * (HD + 1):(t + 1) * (HD + 1)],
                rhs=p[:, n_ * 512:(n_ + 1) * 512],
                start=(t == 0),
                stop=(t == KT - 1),
            )

    def emit_y_pair(n2, m0, evac):
        # Row-tiled pair: block m0 on PE rows 0:63, block m0+1 on 64:127,
        # chunk-interleaved so the two blocks execute concurrently.
        ya = ps_a.tile([128, 1024], FP32, tag="big")
        yb = ps_a.tile([128, 1024], FP32, tag="big")
        for n_ in range(2):
            cols = slice(n2 * QL + n_ * 512, n2 * QL + (n_ + 1) * 512)
            nc.tensor.matmul(
                ya[:, n_ * 512:(n_ + 1) * 512],
                lhsT=img_sb[0:64, WOO + m0 * 128:WOO + (m0 + 1) * 128],
                rhs=oT_sb[0:64, cols],
                start=True,
                stop=True,
            )
            nc.tensor.matmul(
                yb[:, n_ * 512:(n_ + 1) * 512],
                lhsT=img_sb[64:128, WOO + (m0 + 1) * 128:WOO + (m0 + 2) * 128],
                rhs=oB_sb[64:128, cols],
                start=True,
                stop=True,
            )
        for m, yt in ((m0, ya), (m0 + 1, yb)):
            y_sb = ypool.tile([128, 1024], FP16, tag="ysb")
            if evac == "v":
                nc.vector.tensor_copy(y_sb, yt)
                nc.sync.dma_start(
                    out=ypT[m * 128:(m + 1) * 128, n2 * QL:(n2 + 1) * QL],
                    in_=y_sb)
            else:
                # tail: 512-col evac+DMA pipeline across both engines
                nc.scalar.copy(y_sb[:, 0:512], yt[:, 0:512])
                nc.sync.dma_start(
                    out=ypT[m * 128:(m + 1) * 128, n2 * QL:n2 * QL + 512],
                    in_=y_sb[:, 0:512])
                nc.vector.tensor_copy(y_sb[:, 512:1024], yt[:, 512:1024])
                nc.sync.dma_start(
                    out=ypT[m * 128:(m + 1) * 128, n2 * QL + 512:(n2 + 1) * QL],
                    in_=y_sb[:, 512:1024])

    # ---- qc = 0 ---------------------------------------------------------
    # V': [k-tile 128, 65] per tile (col 64 = ones). Uses the ps_o bank so
    # the S-slot rotation keeps 3 buffers. The j2/j3-dependent half (m>=8)
    # is emitted AFTER S-pair(2,3) so its DMA wait cannot block the second
    # pair of exps in the in-order PE queue.
    def emit_v(vp, m0, m1):
        for m in range(m0, m1):
            for c in range(CH):
                base = XO + (m // 4) * 2048 + c * 512 + (m % 4) * 128
                nc.tensor.matmul(
                    vp[:, m * HD:(m + 1) * HD],
                    lhsT=img_sb[:, base:base + 128],
                    rhs=img_sb[:, WVO + c * HD:WVO + (c + 1) * HD],
                    start=(c == 0),
                    stop=(c == CH - 1),
                )

    emit_s_pair(0, 0)
    vp = ps_o.tile([128, 1024], FP32, tag="o")
    emit_v(vp, 0, 8)
    emit_s_pair(0, 2)
    emit_v(vp, 8, 16)
    nc.vector.tensor_copy(
        v_sb.rearrange("p (t c) -> p t c", c=HD + 1)[:, :, 0:HD],
        vp.rearrange("p (t c) -> p t c", c=HD),
    )
    ot = ps_o.tile([128, 1024], FP32, tag="o")
    for t in range(KT):
        if t % 2 == 0:
            if t + 4 < KT:
                if t > 0:
                    emit_s_pair(0, t + 2)
            elif t == 12:
                emit_s_pair(0, 14)
            elif t == 14:
                emit_s_pair(1, 0)   # hoist qc=1's first S pair so the exp
                                    # stream crosses the qc boundary cleanly
        if t in (4, 6, 8, 10):
            emit_w(0, t // 2 + 2)
        if t in (12, 14):
            emit_w(1, t // 2 - 6)
        emit_xpv(0, t, ot)
        if t == 4:
            emit_qk_half(1, startup=False)
    # O'(qc=0) -> SBUF on DVE (ACT stays exp-saturated); dup for Y pairs.
    nc.vector.tensor_copy(oT_sb[:, 0:QL], ot[0:HD + 1, :])
    nc.vector.tensor_copy(oB_sb[64:128, 0:QL], oT_sb[0:64, 0:QL])
    nc.sync.dma_start(out=zrow.rearrange("(a n) -> a n", a=1)[:, 0:QL],
                      in_=oT_sb[HD:HD + 1, 0:QL])

    # ---- qc = 1 (Y pairs of qc=0 hidden in this loop's shadow) ----------
    ot = ps_o.tile([128, 1024], FP32, tag="o")
    for t in range(KT):
        if t % 2 == 0 and t + 2 < KT:
            emit_s_pair(1, t + 2)
        if t in (2, 4, 6, 8, 10, 12):
            emit_w(1, t // 2 + 1)
        emit_xpv(1, t, ot)
        if t == 4:
            emit_y_pair(0, 0, evac="v")
        if t == 10:
            emit_y_pair(0, 2, evac="v")

    # ---- tail: O'(qc=1) evac split across both engines at 512-col
    # granularity so the first Y matmuls start half a copy earlier --------
    nc.scalar.copy(oT_sb[:, QL:QL + 512], ot[0:HD + 1, 0:512])
    nc.vector.tensor_copy(oT_sb[:, QL + 512:N], ot[0:HD + 1, 512:1024])
    nc.vector.tensor_copy(oB_sb[64:128, QL:N], oT_sb[0:64, QL:N])
    nc.sync.dma_start(out=zrow.rearrange("(a n) -> a n", a=1)[:, QL:N],
                      in_=oT_sb[HD:HD + 1, QL:N])
    emit_y_pair(1, 0, evac="sv")
    emit_y_pair(1, 2, evac="sv")


def _install_ntff_hook():
    """Recreate the missing ``antenv.axon_hooks`` module so that
    run_bass_kernel_spmd(trace=True) can capture NTFF profiles via the
    libaxon_pjrt.so ctypes hook (see trn_agent_boot.trn_boot)."""
    import sys
    import types

    try:
        import antenv.axon_hooks  # noqa: F401
        return
    except ImportError:
        pass
    import antenv
    from trn_agent_boot.trn_boot import _ntff_profile_via_ctypes

    mod = types.ModuleType("antenv.axon_hooks")
    mod._hook = _ntff_profile_via_ctypes("/opt/axon/libaxon_pjrt.so")
    mod.set_axon_ntff_profile_hook = lambda h: setattr(mod, "_hook", h)
    mod.get_axon_ntff_profile_hook = lambda: mod._hook
    sys.modules["antenv.axon_hooks"] = mod
    antenv.axon_hooks = mod
    # keep profile artifacts local; the sandbox has no bucket access
    bass_utils.upload_artifacts = lambda tmpdir: tmpdir


def kernel(x, z_matrix, Wq, bq, Wk, bk, Wv, bv, Wo, bo, z_table, _trace=False):
    if _trace:
        _install_ntff_hook()
    x = np.ascontiguousarray(np.asarray(x, dtype=np.float32))
    z_matrix = np.asarray(z_matrix, dtype=np.float32)
    Wq = np.asarray(Wq, dtype=np.float32)
    Wk = np.asarray(Wk, dtype=np.float32)
    Wv = np.asarray(Wv, dtype=np.float32)
    Wo = np.asarray(Wo, dtype=np.float32)
    bq = np.asarray(bq, dtype=np.float32)
    bk = np.asarray(bk, dtype=np.float32)  # cancels in softmax; unused
    bv = np.asarray(bv, dtype=np.float32)
    bo = np.asarray(bo, dtype=np.float32)
    z_table = np.asarray(z_table, dtype=np.float32)

    nc = _build_program()

    xT_img = (
        x.T.astype(BF16_NP)
        .reshape(CH, 128, 4, 512)
        .transpose(1, 2, 0, 3)
        .reshape(128, 8192)
    )
    binsT = np.clip(
        np.floor(z_matrix.T / MAX_Z * NUM_Z_BINS).astype(np.int32), 0, NUM_Z_BINS - 1
    )
    exp_tab = np.exp(z_table)  # [16, H] fp32

    in_maps = []
    for h in range(NCORES):
        sl = slice(h * HD, (h + 1) * HD)
        img = np.zeros((128, IMG_COLS), dtype=BF16_NP)
        img[:, XO:XO + 8192] = xT_img
        wqk = np.concatenate([Wq[:, sl] * SCALE, Wk[:, sl]], axis=1)
        img[:, WQKO:WQKO + 512] = (
            wqk.astype(BF16_NP).reshape(CH, 128, 128)
            .transpose(1, 0, 2).reshape(128, 512)
        )
        img[:, WVO:WVO + 256] = (
            Wv[:, sl].astype(BF16_NP).reshape(CH, 128, HD)
            .transpose(1, 0, 2).reshape(128, 256)
        )
        img[0:64, WOO:WOO + 512] = Wo[sl].astype(BF16_NP)
        img[64:128, WOO:WOO + 512] = Wo[sl].astype(BF16_NP)
        img[0:64, BQO] = (bq[sl] * SCALE).astype(BF16_NP)
        wt_h = exp_tab[:, h][binsT].astype(FP16_NP)  # [key, query] layout
        wt_in = (
            wt_h.reshape(KT, 128, QC, 1024)
            .transpose(2, 1, 0, 3).reshape(QC * 128, KT * 1024)
        )
        in_maps.append({
            "img": img,
            "wt": np.ascontiguousarray(wt_in),
        })

    res = bass_utils.run_bass_kernel_spmd(
        nc, in_maps, core_ids=list(range(NCORES)), trace=_trace,
    )

    acc = np.zeros((D, N), dtype=np.float64)
    for h in range(NCORES):
        ypT_h = res.results[h]["ypT"].astype(np.float64)
        z_h = res.results[h]["zrow"].astype(np.float64)
        acc += ypT_h / z_h[None, :]
    out = acc.T + (bv @ Wo)[None, :] + bo[None, :]
    out_f32 = out.astype(np.float32)
    if _trace:
        return out_f32, res
    return out_f32


# revision 29
# speedup vs baseline: 1.3684x; 1.2358x over previous
"""Graphormer attention (N=2048, D=512, H=8 heads of 64) on 8 NeuronCores.

Strategy (tensor-parallel over heads, one head per core):
  - Host packs x^T + all per-head weights into ONE contiguous DRAM image so
    startup needs 5 big DMA triggers instead of 26 small ones (each trigger
    costs ~600ns of serial sync-queue time).
  - The z-bin bias is folded in multiplicatively: host precomputes
    W = exp(z_table[bin(z)]) in the kernel's [key, query] layout, pre-packed
    per query-chunk so each DMA is contiguous (4KB/partition lines).
  - The K-projection bias bk only adds a per-query constant to scores
    (q . bk), which softmax cancels exactly -> dropped. bq is kept (its
    bq . k_m term varies across keys). SCALE is folded into Wq/bq on host.
  - On device (per core): fused Q^T/K^T projection (one [128,128] weight
    block -> Q rows 0:64, K rows 64:128), evacuated per 512-col chunk with
    the two evacs split across ScalarE/VectorE so S can start early.
  - Main loop per k-tile (baseline's proven in-order pattern -- S(t+2)
    emitted before exp/mult/PV(t) so a stalled PV never starves the next
    exp): S^T = K^T x Q^T (fp32, PSUM) -> exp on ScalarE (the bottleneck:
    32 x [128,1024] @ ~1.0us) -> P = exp(S)*W on VectorE -> O'^T +=
    V'[128,65] x P (65th V column = ones => row 64 of O' = softmax denom Z).
  - V projection runs through the ps_o PSUM bank before the k-tile loop;
    QK half-1 is interleaved mid-loop (DVE evacs; ACT stays exp-saturated);
    Y^T(qc=0) projection blocks hide inside qc=1's loop shadow.
  - Host divides each head's partial Y by its Z, sums heads, adds biases.
"""

import numpy as np
import ml_dtypes
from contextlib import ExitStack

import concourse.bass as bass
import concourse.tile as tile
from concourse import bacc, mybir
from concourse import bass_utils

N = 2048
D = 512
H = 8
HD = 64
NUM_Z_BINS = 16
MAX_Z = 5.0
SCALE = HD ** -0.5
NCORES = 8
QL = 1024          # query-chunk length (PSUM budget)
QC = N // QL       # 2 query chunks
KT = N // 128      # 16 key tiles
CH = D // 128      # 4 contraction chunks of the model dim

# packed image column offsets (bf16, [128, IMG_COLS])
XO = 0             # x^T, col-chunked: [j, c, u] -> j*2048 + c*512 + u
WQKO = 8192        # [Wq*SCALE | Wk] chunks: c*128 + v
BQO = 8704         # bq*SCALE, rows 0:64 (avoids a tiny separate DMA whose
                   # 4-byte descriptors complete ~10us late behind big ones)
WVO = 8708         # Wv chunks: c*64 + v
WOO = 8964         # Wo rows 0:64 (rows 64:128 unused)
IMG_COLS = 9476

FP32 = mybir.dt.float32
FP16 = mybir.dt.float16
BF16 = mybir.dt.bfloat16
BF16_NP = ml_dtypes.bfloat16
FP16_NP = np.float16

AF = mybir.ActivationFunctionType
OP = mybir.AluOpType

_PROGRAM_CACHE = {}


def _build_program():
    if "nc" in _PROGRAM_CACHE:
        return _PROGRAM_CACHE["nc"]

    nc = bacc.Bacc(
        "TRN2",
        target_bir_lowering=False,
        debug=False,
        enable_asserts=False,
        num_devices=NCORES,
    )

    img = nc.dram_tensor("img", [128, IMG_COLS], BF16, kind="ExternalInput").ap()
    wt = nc.dram_tensor("wt", [QC * 128, KT * 1024], FP16, kind="ExternalInput").ap()

    ypT = nc.dram_tensor("ypT", [D, N], FP16, kind="ExternalOutput").ap()
    zrow = nc.dram_tensor("zrow", [N], FP16, kind="ExternalOutput").ap()

    with tile.TileContext(nc) as tc:
        with ExitStack() as ctx:
            _emit(ctx, tc, img, wt, ypT, zrow)
    nc.compile()
    _PROGRAM_CACHE["nc"] = nc
    return nc


def _emit(ctx, tc, img, wt, ypT, zrow):
    nc = tc.nc

    singles = ctx.enter_context(tc.tile_pool(name="singles", bufs=1))
    # PSUM: ps_a 3 x [128,1024]f32 (2 banks each) shared by warmup/qk/s/y;
    # ps_o 1 x [128,1024]f32 (2 banks) holds V' then the O' accumulator.
    ps_a = ctx.enter_context(tc.tile_pool(name="ps_a", bufs=3, space="PSUM"))
    ps_o = ctx.enter_context(tc.tile_pool(name="ps_o", bufs=1, space="PSUM"))
    wpool = ctx.enter_context(tc.tile_pool(name="wpool", bufs=4))
    epool = ctx.enter_context(tc.tile_pool(name="epool", bufs=4))
    ppool = ctx.enter_context(tc.tile_pool(name="ppool", bufs=3))
    ypool = ctx.enter_context(tc.tile_pool(name="ypool", bufs=4))

    w_tiles = {}

    def emit_w(qc, i):
        w = wpool.tile([128, 2048], FP16, tag="w")
        nc.sync.dma_start(
            out=w, in_=wt[qc * 128:(qc + 1) * 128, i * 2048:(i + 1) * 2048])
        w_tiles[(qc, i)] = w

    # ---- input DMAs. The QK weights ride the Scalar queue (also HWDGE)
    # so they stream concurrently with the x^T chunks on the Sync queue;
    # order within each queue = completion priority (exp stream needs
    # wqk+bias, j0, j1 first).
    img_sb = singles.tile([128, IMG_COLS], BF16)
    nc.scalar.dma_start(out=img_sb[:, WQKO:WVO], in_=img[:, WQKO:WVO])
    nc.sync.dma_start(out=img_sb[:, 0:2048], in_=img[:, 0:2048])
    nc.scalar.dma_start(out=img_sb[:, WVO:IMG_COLS], in_=img[:, WVO:IMG_COLS])
    nc.sync.dma_start(out=img_sb[:, 2048:4096], in_=img[:, 2048:4096])
    nc.sync.dma_start(out=img_sb[:, 4096:6144], in_=img[:, 4096:6144])
    nc.sync.dma_start(out=img_sb[:, 6144:8192], in_=img[:, 6144:8192])
    emit_w(0, 0)
    emit_w(0, 1)
    emit_w(0, 2)
    emit_w(0, 3)
    bq_sb = singles.tile([HD, 1], FP32)
    nc.vector.tensor_copy(bq_sb, img_sb[0:64, BQO:BQO + 1])

    # ---- warmup: prime the ACT table load + keep the PE HAM busy -------
    scratch = singles.tile([128, 512], BF16)
    nc.vector.memset(scratch, 0.0)
    v_sb = singles.tile([128, KT * (HD + 1)], FP16)
    nc.vector.memset(v_sb, 1.0)  # col 64 of each V' tile stays 1.0 (Z row)
    warm16 = singles.tile([1, 16], FP16)
    nc.scalar.activation(warm16, scratch[0:1, 0:16], AF.Exp)  # table load now
    wu = ps_a.tile([128, 1024], FP32, tag="big")
    for _ in range(4):
        nc.tensor.matmul(wu[:, 0:512], lhsT=scratch[:, 0:128], rhs=scratch,
                         start=True, stop=True)

    # q2/k2 hold Q^T/K^T duplicated in BOTH partition halves so the K=64
    # S^T matmuls can ROW-TILE: even k-tiles use PE rows 0:63, odd tiles
    # rows 64:127 -- adjacent tiles execute concurrently when the PE is the
    # laggard (i.e. exactly in the clock-throttled regime).
    q2 = singles.tile([128, N], BF16)
    k2 = singles.tile([128, N], BF16)
    oT_sb = singles.tile([HD + 1, N], FP16)  # rows 0:65 = [O'; Z]
    oB_sb = singles.tile([128, N], FP16)     # rows 64:128 = O' copy (Y pairs)

    def emit_qk_half(half, startup):
        pt = ps_a.tile([128, 1024], FP32, tag="big")
        for n_ in range(2):
            j = half * 2 + n_
            for c in range(CH):
                nc.tensor.matmul(
                    pt[:, n_ * 512:(n_ + 1) * 512],
                    lhsT=img_sb[:, WQKO + c * 128:WQKO + (c + 1) * 128],
                    rhs=img_sb[:, XO + j * 2048 + c * 512:XO + j * 2048 + (c + 1) * 512],
                    start=(c == 0),
                    stop=(c == CH - 1),
                )
            if startup:
                # 512-col granularity, split across both engines so the
                # first S matmuls can start as early as possible.
                src = slice(n_ * 512, (n_ + 1) * 512)
                dst = slice(half * 1024 + n_ * 512, half * 1024 + (n_ + 1) * 512)
                # lo-half copies first on both engines: S(0) runs on the lo
                # half, so q-lo/k-lo gate the first exp.
                nc.vector.tensor_scalar(q2[0:64, dst], pt[0:64, src], bq_sb,
                                        None, OP.add)
                nc.scalar.copy(k2[0:64, dst], pt[64:128, src])
                nc.vector.tensor_scalar(q2[64:128, dst], pt[0:64, src], bq_sb,
                                        None, OP.add)
                nc.scalar.copy(k2[64:128, dst], pt[64:128, src])
        if not startup:
            # mid-loop: ACT is exp-saturated -> all four evacs on DVE,
            # straight from PSUM (SBUF->SBUF DMA dups added ~3us of latency
            # to the S(8) critical path). lo-half first: S(8) runs on lo.
            dst = slice(half * 1024, (half + 1) * 1024)
            nc.vector.tensor_scalar(q2[0:64, dst], pt[0:64, :], bq_sb,
                                    None, OP.add)
            nc.vector.tensor_copy(k2[0:64, dst], pt[64:128, :])
            nc.vector.tensor_copy(k2[64:128, dst], pt[64:128, :])
            nc.vector.tensor_scalar(q2[64:128, dst], pt[0:64, :], bq_sb,
                                    None, OP.add)

    emit_qk_half(0, startup=True)

    # ---- main-loop emission helpers ------------------------------------
    s_tiles = {}

    def emit_s_pair(qc, t0):
        # Interleave the two tiles' 512-chunks so their row groups overlap.
        sa = ps_a.tile([128, 1024], FP32, tag="big")
        sb = ps_a.tile([128, 1024], FP32, tag="big")
        for n_ in range(2):
            nc.tensor.matmul(
                sa[:, n_ * 512:(n_ + 1) * 512],
                lhsT=k2[0:64, t0 * 128:(t0 + 1) * 128],
                rhs=q2[0:64, qc * QL + n_ * 512:qc * QL + (n_ + 1) * 512],
                start=True,
                stop=True,
            )
            nc.tensor.matmul(
                sb[:, n_ * 512:(n_ + 1) * 512],
                lhsT=k2[64:128, (t0 + 1) * 128:(t0 + 2) * 128],
                rhs=q2[64:128, qc * QL + n_ * 512:qc * QL + (n_ + 1) * 512],
                start=True,
                stop=True,
            )
        s_tiles[(qc, t0)] = sa
        s_tiles[(qc, t0 + 1)] = sb

    def emit_xpv(qc, t, ot):
        st = s_tiles.pop((qc, t))
        e = epool.tile([128, 1024], FP16, tag="e")
        nc.scalar.activation(e, st, AF.Exp)
        p = ppool.tile([128, 1024], FP16, tag="p")
        w = w_tiles[(qc, t // 2)]
        nc.vector.tensor_mul(p, e, w[:, (t % 2) * 1024:(t % 2 + 1) * 1024])
        for n_ in range(2):
            nc.tensor.matmul(
                ot[0:HD + 1, n_ * 512:(n_ + 1) * 512],
                lhsT=v_sb[:, t * (HD + 1):(t + 1) * (HD + 1)],
                rhs=p[:, n_ * 512:(n_ + 1) * 512],
                start=(t == 0),
                stop=(t == KT - 1),
            )

    def emit_y_pair(n2, m0, evac):
        # Row-tiled pair: block m0 on PE rows 0:63, block m0+1 on 64:127,
        # chunk-interleaved so the two blocks execute concurrently.
        ya = ps_a.tile([128, 1024], FP32, tag="big")
        yb = ps_a.tile([128, 1024], FP32, tag="big")
        for n_ in range(2):
            cols = slice(n2 * QL + n_ * 512, n2 * QL + (n_ + 1) * 512)
            nc.tensor.matmul(
                ya[:, n_ * 512:(n_ + 1) * 512],
                lhsT=img_sb[0:64, WOO + m0 * 128:WOO + (m0 + 1) * 128],
                rhs=oT_sb[0:64, cols],
                start=True,
                stop=True,
            )
            nc.tensor.matmul(
                yb[:, n_ * 512:(n_ + 1) * 512],
                lhsT=img_sb[64:128, WOO + (m0 + 1) * 128:WOO + (m0 + 2) * 128],
                rhs=oB_sb[64:128, cols],
                start=True,
                stop=True,
            )
        for m, yt in ((m0, ya), (m0 + 1, yb)):
            y_sb = ypool.tile([128, 1024], FP16, tag="ysb")
            if evac == "v":
                nc.vector.tensor_copy(y_sb, yt)
                nc.sync.dma_start(
                    out=ypT[m * 128:(m + 1) * 128, n2 * QL:(n2 + 1) * QL],
                    in_=y_sb)
            else:
                # tail: 512-col evac+DMA pipeline across both engines
                nc.scalar.copy(y_sb[:, 0:512], yt[:, 0:512])
                nc.sync.dma_start(
                    out=ypT[m * 128:(m + 1) * 128, n2 * QL:n2 * QL + 512],
                    in_=y_sb[:, 0:512])
                nc.vector.tensor_copy(y_sb[:, 512:1024], yt[:, 512:1024])
                nc.sync.dma_start(
                    out=ypT[m * 128:(m + 1) * 128, n2 * QL + 512:(n2 + 1) * QL],
                    in_=y_sb[:, 512:1024])

    # ---- qc = 0 ---------------------------------------------------------
    # V': [k-tile 128, 65] per tile (col 64 = ones). Uses the ps_o bank so
    # the S-slot rotation keeps 3 buffers. The j2/j3-dependent half (m>=8)
    # is emitted AFTER S-pair(2,3) so its DMA wait cannot block the second
    # pair of exps in the in-order PE queue.
    def emit_v(vp, m0, m1):
        for m in range(m0, m1):
            for c in range(CH):
                base = XO + (m // 4) * 2048 + c * 512 + (m % 4) * 128
                nc.tensor.matmul(
                    vp[:, m * HD:(m + 1) * HD],
                    lhsT=img_sb[:, base:base + 128],
                    rhs=img_sb[:, WVO + c * HD:WVO + (c + 1) * HD],
                    start=(c == 0),
                    stop=(c == CH - 1),
                )

    emit_s_pair(0, 0)
    vp = ps_o.tile([128, 1024], FP32, tag="o")
    emit_v(vp, 0, 8)
    emit_s_pair(0, 2)
    emit_v(vp, 8, 16)
    nc.vector.tensor_copy(
        v_sb.rearrange("p (t c) -> p t c", c=HD + 1)[:, :, 0:HD],
        vp.rearrange("p (t c) -> p t c", c=HD),
    )
    ot = ps_o.tile([128, 1024], FP32, tag="o")
    for t in range(KT):
        if t % 2 == 0:
            if t + 4 < KT:
                if t > 0:
                    emit_s_pair(0, t + 2)
            elif t == 12:
                emit_s_pair(0, 14)
            elif t == 14:
                emit_s_pair(1, 0)   # hoist qc=1's first S pair so the exp
                                    # stream crosses the qc boundary cleanly
        if t in (4, 6, 8, 10):
            emit_w(0, t // 2 + 2)
        if t in (12, 14):
            emit_w(1, t // 2 - 6)
        emit_xpv(0, t, ot)
        if t == 2:
            emit_qk_half(1, startup=False)
    # O'(qc=0) -> SBUF on DVE (ACT stays exp-saturated); dup for Y pairs.
    nc.vector.tensor_copy(oT_sb[:, 0:QL], ot[0:HD + 1, :])
    nc.vector.tensor_copy(oB_sb[64:128, 0:QL], oT_sb[0:64, 0:QL])
    nc.sync.dma_start(out=zrow.rearrange("(a n) -> a n", a=1)[:, 0:QL],
                      in_=oT_sb[HD:HD + 1, 0:QL])

    # ---- qc = 1 (Y pairs of qc=0 hidden in this loop's shadow) ----------
    ot = ps_o.tile([128, 1024], FP32, tag="o")
    for t in range(KT):
        if t % 2 == 0 and t + 2 < KT:
            emit_s_pair(1, t + 2)
        if t in (2, 4, 6, 8, 10, 12):
            emit_w(1, t // 2 + 1)
        emit_xpv(1, t, ot)
        if t == 4:
            emit_y_pair(0, 0, evac="v")
        if t == 10:
            emit_y_pair(0, 2, evac="v")

    # ---- tail: O'(qc=1) evac split across both engines at 512-col
    # granularity so the first Y matmuls start half a copy earlier --------
    nc.scalar.copy(oT_sb[:, QL:QL + 512], ot[0:HD + 1, 0:512])
    nc.vector.tensor_copy(oT_sb[:, QL + 512:N], ot[0:HD + 1, 512:1024])
    nc.vector.tensor_copy(oB_sb[64:128, QL:N], oT_sb[0:64, QL:N])
    nc.sync.dma_start(out=zrow.rearrange("(a n) -> a n", a=1)[:, QL:N],
                      in_=oT_sb[HD:HD + 1, QL:N])
    emit_y_pair(1, 0, evac="sv")
    emit_y_pair(1, 2, evac="sv")


def _install_ntff_hook():
    """Recreate the missing ``antenv.axon_hooks`` module so that
    run_bass_kernel_spmd(trace=True) can capture NTFF profiles via the
    libaxon_pjrt.so ctypes hook (see trn_agent_boot.trn_boot)."""
    import sys
    import types

    try:
        import antenv.axon_hooks  # noqa: F401
        return
    except ImportError:
        pass
    import antenv
    from trn_agent_boot.trn_boot import _ntff_profile_via_ctypes

    mod = types.ModuleType("antenv.axon_hooks")
    mod._hook = _ntff_profile_via_ctypes("/opt/axon/libaxon_pjrt.so")
    mod.set_axon_ntff_profile_hook = lambda h: setattr(mod, "_hook", h)
    mod.get_axon_ntff_profile_hook = lambda: mod._hook
    sys.modules["antenv.axon_hooks"] = mod
    antenv.axon_hooks = mod
    # keep profile artifacts local; the sandbox has no bucket access
    bass_utils.upload_artifacts = lambda tmpdir: tmpdir


def kernel(x, z_matrix, Wq, bq, Wk, bk, Wv, bv, Wo, bo, z_table, _trace=False):
    if _trace:
        _install_ntff_hook()
    x = np.ascontiguousarray(np.asarray(x, dtype=np.float32))
    z_matrix = np.asarray(z_matrix, dtype=np.float32)
    Wq = np.asarray(Wq, dtype=np.float32)
    Wk = np.asarray(Wk, dtype=np.float32)
    Wv = np.asarray(Wv, dtype=np.float32)
    Wo = np.asarray(Wo, dtype=np.float32)
    bq = np.asarray(bq, dtype=np.float32)
    bk = np.asarray(bk, dtype=np.float32)  # cancels in softmax; unused
    bv = np.asarray(bv, dtype=np.float32)
    bo = np.asarray(bo, dtype=np.float32)
    z_table = np.asarray(z_table, dtype=np.float32)

    nc = _build_program()

    xT_img = (
        x.T.astype(BF16_NP)
        .reshape(CH, 128, 4, 512)
        .transpose(1, 2, 0, 3)
        .reshape(128, 8192)
    )
    binsT = np.clip(
        np.floor(z_matrix.T / MAX_Z * NUM_Z_BINS).astype(np.int32), 0, NUM_Z_BINS - 1
    )
    exp_tab = np.exp(z_table)  # [16, H] fp32

    in_maps = []
    for h in range(NCORES):
        sl = slice(h * HD, (h + 1) * HD)
        img = np.zeros((128, IMG_COLS), dtype=BF16_NP)
        img[:, XO:XO + 8192] = xT_img
        wqk = np.concatenate([Wq[:, sl] * SCALE, Wk[:, sl]], axis=1)
        img[:, WQKO:WQKO + 512] = (
            wqk.astype(BF16_NP).reshape(CH, 128, 128)
            .transpose(1, 0, 2).reshape(128, 512)
        )
        img[:, WVO:WVO + 256] = (
            Wv[:, sl].astype(BF16_NP).reshape(CH, 128, HD)
            .transpose(1, 0, 2).reshape(128, 256)
        )
        img[0:64, WOO:WOO + 512] = Wo[sl].astype(BF16_NP)
        img[64:128, WOO:WOO + 512] = Wo[sl].astype(BF16_NP)
        img[0:64, BQO] = (bq[sl] * SCALE).astype(BF16_NP)
        wt_h = exp_tab[:, h][binsT].astype(FP16_NP)  # [key, query] layout
        wt_in = (
            wt_h.reshape(KT, 128, QC, 1024)
            .transpose(2, 1, 0, 3).reshape(QC * 128, KT * 1024)
        )
        in_maps.append({
            "img": img,
            "wt": np.ascontiguousarray(wt_in),
        })

    res = bass_utils.run_bass_kernel_spmd(
        nc, in_maps, core_ids=list(range(NCORES)), trace=_trace,
    )

    acc = np.zeros((D, N), dtype=np.float64)
    for h in range(NCORES):
        ypT_h = res.results[h]["ypT"].astype(np.float64)
        z_h = res.results[h]["zrow"].astype(np.float64)
        acc += ypT_h / z_h[None, :]
    out = acc.T + (bv @ Wo)[None, :] + bo[None, :]
    out_f32 = out.astype(np.float32)
    if _trace:
        return out_f32, res
    return out_f32
